# revision 5
# baseline (speedup 1.0000x reference)
"""MEX (log-sum-exp) 3x3 pooling kernel for Trainium2, 8-core SPMD.

Math: out[b,m,i,j] = log( (1/n) * sum_{c,dh,dw} exp(x[b,c,i+dh,j+dw] + off[m,c,dh,dw]) )
with n = C*3*3 = 576, eps = 1.

Identity used: the reference's per-pixel max-stabilization cancels exactly:
  out = m_x + m_b + log(S) - log(n)  ==  log( sum_k exp(x_k + b_k) ) - log(n)
Values are benign in fp32 (x ~ N(0,1) -> exp(x) in [4e-3, 260]; off = log_softmax
values in [-13, -2.5] -> exp(off) in [2e-6, 0.08]), so no stabilization is needed.

Per-core plan (core i handles batch images 2i, 2i+1):
  - SBUF E tile (128, 16384+pad): partition p = img*64 + c, free = h*128 + w,
    E = exp(x) computed by ACT in chunks as DMA streams x in.
  - Weights: off permuted host-side to wp[c, (dh,dw,m)] (64,144); device computes
    exp and scatters into lhsT layout LT[img*64+c, dh*96 + dw*32 + img*16 + m],
    zero elsewhere (block-diagonal over img so the two images stay independent).
  - For each 2048-pixel superchunk: 3 PSUM-accumulated matmuls (over dh, each
    N=512 x 4 psum banks) with rhs = E slice at offset dh*128.  Output psum
    P[(dw,img,m), n] holds per-dw-tap partial sums.
  - Two DVE shifted adds combine the dw groups:
      OUT[img*16+m, j] = P[p,j] + P[32+p, j+1] + P[64+p, j+2]
  - ACT computes log(OUT * 1/576) and a strided DMA writes the valid
    (row, col<=125) region to DRAM.
"""

import numpy as np

EPS = 1.0
B, C, H, W = 16, 64, 128, 128
M = 16
BH = BW = 3
HO, WO = H - BH + 1, W - BW + 1  # 126, 126
N_TAPS = C * BH * BW  # 576
NCORES = 8
BPC = B // NCORES  # 2 images per core
HWP = H * W  # 16384 pixels per image plane
PAD = 768
SC = 2048  # superchunk pixels (4 psum banks of 512 fp32)
NSC = HWP // SC  # 8
DMACH = 1024  # x DMA / exp chunk size in pixels
NDMACH = HWP // DMACH

# matmul dtype: "f32r" = single-pass fp32 (full rate, slightly reduced HW
# precision), "f32" = exact fp32 (4 cycles/row).
import os as _os

MM_DTYPE = _os.environ.get("MEX_MM_DTYPE", "f32r")

_BUILT = {}


def _build(mm_dtype: str):
    """Build (and cache) the Bass/Tile program shared by all 8 cores."""
    if mm_dtype in _BUILT:
        return _BUILT[mm_dtype]

    import concourse.bass as bass
    import concourse.bacc as bacc
    import concourse.tile as tile
    from concourse import mybir

    f32 = mybir.dt.float32
    # dtype of matmul operand tiles: walrus requires producers of f32r matmul
    # operands to emit f32r-typed outputs, so E/QE/LT are natively f32r.
    mdt = mybir.dt.float32r if mm_dtype == "f32r" else f32
    AF = mybir.ActivationFunctionType

    nc = bacc.Bacc("TRN2", target_bir_lowering=False, debug=False)

    xd = nc.dram_tensor("x", [128, HWP], f32, kind="ExternalInput")
    wpd = nc.dram_tensor("wp", [64, 144], f32, kind="ExternalInput")
    outd = nc.dram_tensor("out", [BPC * M, HO * WO], f32, kind="ExternalOutput")

    with tile.TileContext(nc) as tc:
        with (
            tc.tile_pool(name="singles", bufs=1) as singles,
            tc.tile_pool(name="xin", bufs=4) as xin,
            tc.tile_pool(name="psum", bufs=2, space="PSUM") as psum,
            tc.tile_pool(name="post", bufs=3) as post,
        ):
            # ---- weights: wp -> scatter into pre-exp lhsT layout -> exp ----
            # (memset is not legal on f32r tiles, so build an f32 "log-domain"
            # LT with -80 in the zero slots and take exp through ACT, which
            # legally produces f32r: exp(-80) ~ 2e-35 is negligible vs S>=1e-3.)
            Q = singles.tile([128, 144], f32)
            nc.sync.dma_start(out=Q[0:64, :], in_=bass.AP(wpd, 0, [[144, 64], [1, 144]]))
            nc.sync.dma_start(out=Q[64:128, :], in_=bass.AP(wpd, 0, [[144, 64], [1, 144]]))

            QL = singles.tile([128, 288], f32)
            nc.vector.memset(QL[:, :], -80.0)
            QLv = QL[:, :].rearrange("p (dh dw i m) -> p dh dw i m", dh=3, dw=3, i=2)
            Qv = Q[:, :].rearrange("p (dh dw m) -> p dh dw m", dh=3, dw=3)
            nc.vector.tensor_copy(out=QLv[0:64, :, :, 0, :], in_=Qv[0:64])
            nc.vector.tensor_copy(out=QLv[64:128, :, :, 1, :], in_=Qv[64:128])
            LT = singles.tile([128, 288], mdt)
            nc.scalar.activation(out=LT[:, :], in_=QL[:, :], func=AF.Exp, scale=EPS)

            # ---- E = exp(x), streamed; pad gets exp(0)=1 ----
            E = singles.tile([128, HWP + PAD], mdt)
            Xpad = singles.tile([128, PAD], f32)
            nc.vector.memset(Xpad[:, :], 0.0)
            nc.scalar.activation(
                out=E[:, HWP:], in_=Xpad[:, :], func=AF.Exp, scale=EPS
            )
            for k in range(NDMACH):
                Xk = xin.tile([128, DMACH], f32)
                nc.sync.dma_start(
                    out=Xk[:, :],
                    in_=bass.AP(xd, k * DMACH, [[HWP, 128], [1, DMACH]]),
                )
                nc.scalar.activation(
                    out=E[:, k * DMACH : (k + 1) * DMACH],
                    in_=Xk[:, :],
                    func=AF.Exp,
                    scale=EPS,
                )

            # ---- main conv + log loop ----
            LTd = LT[:, :].rearrange("p (dh c) -> p dh c", dh=3)
            for s in range(NSC):
                P = psum.tile([96, SC], f32)
                for dh in range(3):
                    lhsT = LTd[:, dh, :]
                    for b4 in range(SC // 512):
                        base = s * SC + dh * W + b4 * 512
                        nc.tensor.matmul(
                            P[:, b4 * 512 : (b4 + 1) * 512],
                            lhsT,
                            E[:, base : base + 512],
                            start=(dh == 0),
                            stop=(dh == 2),
                        )
                # DVE may read only one PSUM operand per instruction: copy the
                # dw=1 group to SBUF (2x-rate copy), then two 1-psum adds.
                Pc = post.tile([32, SC - 2], f32, tag="Pc")
                nc.vector.tensor_copy(out=Pc[:, :], in_=P[32:64, 1 : SC - 1])
                A = post.tile([32, SC - 2], f32, tag="A")
                nc.vector.tensor_add(
                    out=A[:, :], in0=P[0:32, 0 : SC - 2], in1=Pc[:, :]
                )
                S2 = post.tile([32, SC - 2], f32, tag="S2")
                nc.vector.tensor_add(out=S2[:, :], in0=A[:, :], in1=P[64:96, 2:SC])
                LG = post.tile([32, SC - 2], f32, tag="LG")
                nc.scalar.activation(
                    out=LG[:, :], in_=S2[:, :], func=AF.Ln, scale=1.0 / float(N_TAPS)
                )
                # rows of this superchunk: 16s .. 16s+15; valid rows <= 125
                nrows = min(16, HO - s * 16)
                lg = LG[:, :]
                src = bass.AP(lg.tensor, lg.offset, [lg.ap[0], [W, nrows], [1, WO]])
                dst = bass.AP(
                    outd, s * 16 * WO, [[HO * WO, BPC * M], [WO, nrows], [1, WO]]
                )
                nc.sync.dma_start(out=dst, in_=src)

    nc.compile()
    _BUILT[mm_dtype] = nc
    return nc


def _prep_inputs(x, offsets):
    x = np.ascontiguousarray(np.asarray(x), dtype=np.float32)
    off = np.asarray(offsets, dtype=np.float32).reshape(M, C, BH, BW)
    # wp[c, dh*48 + dw*16 + m] = off[m, c, dh, dw]
    wp = np.ascontiguousarray(np.transpose(off, (1, 2, 3, 0)).reshape(64, 144))
    in_maps = [
        {"x": np.ascontiguousarray(x[BPC * i : BPC * (i + 1)]).reshape(128, HWP), "wp": wp}
        for i in range(NCORES)
    ]
    return in_maps


def kernel(x, offsets):
    from concourse.bass_utils import run_bass_kernel_spmd

    nc = _build(MM_DTYPE)
    in_maps = _prep_inputs(x, offsets)
    res = run_bass_kernel_spmd(nc, in_maps, core_ids=list(range(NCORES)))
    out = np.empty((B, M, HO, WO), dtype=np.float32)
    for i in range(NCORES):
        out[BPC * i : BPC * (i + 1)] = res.results[i]["out"].reshape(BPC, M, HO, WO)
    return out
